# revision 1
# baseline (speedup 1.0000x reference)
"""AttentionTSSA Trainium2 kernel — full-IO contract.

kernel(**inputs) takes the FULL inputs (x [8,512,128,128], qkv_w, temp,
out_w, out_b), shards data-parallel over batch across the 8 NeuronCores
(batch i -> core i), runs a Bass/Tile kernel per core, and returns the
full [8,512,128,128] float32 output.

Per-core computation (one batch, layout [d, n], d on partitions):
  w = qkv_w @ xb                         (fp16 matmuls, PE)
  norm2[d] = sum_n w^2                   (ACT/DVE square + free-axis accum)
  logits[h,n] = sum_d w^2/norm2[d]       (PE, invnorm2-masked lhsT, x256)
  P = exp(temp/256 * logits)             (ACT)
  Pi = exp(temp/256*logits - ln(sum_h P))  (ones-matmul + ACT Ln + DVE sub
                                            + ACT exp; accum gives S[h])
  dots[d] = sum_n Pi_bcast * w^2         (PE indicator-matmul + DVE STT)
  attn = 1/(1+dots/(S+1e-8)), folded into projection weights
  o = w * Pi_bcast                       (DVE)
  y = (out_w * -attn) @ o + out_b        (fp16 matmuls, ACT bias add)

w (fp16) and Pi (fp16) persist in SBUF between the three passes.
After Bacc compile, redundant ACT-table loads are collapsed into a
single natural_log_exp_and_others load (it contains exp/ln/square/
copy/identity — the compiler's per-function greedy pick would other-
wise reload tables twice per tile).
"""

import sys

sys.path.insert(0, "/opt/trn_rl_repo")

from contextlib import ExitStack

import numpy as np

import concourse.bass as bass
import concourse.tile as tile
from concourse import bacc, mybir
from concourse.bass_utils import run_bass_kernel_spmd
from concourse.hw_specs import get_activation_tables
F32 = mybir.dt.float32
F16 = mybir.dt.bfloat16   # averaged paths: sq, lmat
F16A = mybir.dt.float16   # value paths: x, w, P, Pi, o, weights
AF = mybir.ActivationFunctionType
ALU = mybir.AluOpType

B = 8            # batch == number of cores
C = 512          # channels
H_IMG, W_IMG = 128, 128
N = H_IMG * W_IMG
HEADS = 8
HD = 64          # head dim
NT = 512         # tokens per tile
KD = 4           # 128-partition tiles of the channel dim
P = 128
LM_SCALE = 256.0  # keeps invnorm2 out of fp16-subnormal range in lmat

_NC_CACHE = {}


def _dedupe_act_table_loads(nc):
    """Collapse all InstLoadActFuncSet into one load of the set that
    contains every function this kernel uses (exp, ln, square, copy,
    identity). The kernel CFG is a single linear block per engine, so a
    single leading load is sufficient."""
    tables = list(get_activation_tables(nc.m.arch).keys())
    want = {AF.Exp, AF.Ln, AF.Square, AF.Copy, AF.Identity}
    sets = get_activation_tables(nc.m.arch)
    target = None
    for idx, name in enumerate(tables):
        if want <= sets[name]:
            target = idx
            break
    if target is None:
        return
    first = True
    for blk in nc.main_func.blocks:
        keep = []
        for inst in blk.instructions:
            if isinstance(inst, mybir.InstLoadActFuncSet):
                si = inst.sync_info
                has_sync = si is not None and (
                    len(si.on_wait) > 0 or len(si.on_update) > 0)
                if first or has_sync:
                    inst.act_func_set_id = target
                    first = False
                    keep.append(inst)
            else:
                keep.append(inst)
        blk.instructions[:] = keep


def _build_nc(n_tokens=N, n_cores=B):
    NTILES = n_tokens // NT
    nc = bacc.Bacc("TRN2", target_bir_lowering=False, debug=False,
                   num_devices=n_cores)

    xb = nc.dram_tensor("xb", [C, n_tokens], F16A, kind="ExternalInput").ap()
    qkvwT = nc.dram_tensor("qkvwT", [C, C], F16A, kind="ExternalInput").ap()
    outwT = nc.dram_tensor("outwT", [C, C], F16A, kind="ExternalInput").ap()
    ind = nc.dram_tensor("ind", [HEADS, C], F16A, kind="ExternalInput").ap()
    ones8 = nc.dram_tensor("ones8", [HEADS, HEADS], F16A,
                           kind="ExternalInput").ap()
    temp_s = nc.dram_tensor("temp_s", [HEADS, 1], F32,
                            kind="ExternalInput").ap()
    outb = nc.dram_tensor("outb", [C, 1], F32, kind="ExternalInput").ap()
    y = nc.dram_tensor("y", [C, n_tokens], F32, kind="ExternalOutput").ap()
    svec_dram = nc.dram_tensor("svec_scratch", [HEADS, 1], F32).ap()

    with tile.TileContext(nc) as tc, ExitStack() as top:
        const = top.enter_context(tc.tile_pool(name="const", bufs=1))
        persist = top.enter_context(tc.tile_pool(name="persist", bufs=1))

        # --- constants into SBUF -------------------------------------------
        qkvwT_sb = [const.tile([P, C], F16A, name=f"qkvwT{k}") for k in range(KD)]
        outwT_sb = [const.tile([P, C], F16A, name=f"outwT{k}") for k in range(KD)]
        for k in range(KD):
            nc.sync.dma_start(qkvwT_sb[k][:], qkvwT[k * P:(k + 1) * P, :])
            nc.sync.dma_start(outwT_sb[k][:], outwT[k * P:(k + 1) * P, :])
        ind_sb = const.tile([HEADS, C], F16A, name="ind")
        nc.sync.dma_start(ind_sb[:], ind)
        ones8_sb = const.tile([HEADS, HEADS], F16A, name="ones8")
        nc.sync.dma_start(ones8_sb[:], ones8)
        temp_sb = const.tile([HEADS, 1], F32, name="temp")
        nc.sync.dma_start(temp_sb[:], temp_s)
        outb_sb = const.tile([P, KD], F32, name="outb")
        for k in range(KD):
            nc.sync.dma_start(outb_sb[:, k:k + 1], outb[k * P:(k + 1) * P, :])

        # --- persistent state ----------------------------------------------
        w_store = [persist.tile([P, n_tokens], F16A, name=f"w{k}")
                   for k in range(KD)]
        pi_store = persist.tile([HEADS, n_tokens], F16A, name="pi")
        norm2_part = persist.tile([P, KD * NTILES], F32, name="norm2p")
        dots_part = persist.tile([P, KD * NTILES], F32, name="dotsp")
        s_part = persist.tile([HEADS, NTILES], F32, name="sp")
        inv2 = persist.tile([P, KD], F32, name="inv2")
        lmat = persist.tile([P, KD * HEADS], F16, name="lmat")  # logits lhsT
        nattn = persist.tile([P, KD], F32, name="nattn")
        outwA = [persist.tile([P, C], F16A, name=f"outwA{k}")
                 for k in range(KD)]

        # =================== Phase 1: qkv matmul + norm2 ===================
        with ExitStack() as p1:
            xpool = p1.enter_context(tc.tile_pool(name="x", bufs=8))
            sqscr = p1.enter_context(tc.tile_pool(name="sqscr", bufs=2))
            wps = p1.enter_context(tc.tile_pool(name="wps", bufs=6, space="PSUM"))
            for t in range(NTILES):
                xs = []
                for kc in range(KD):
                    xt = xpool.tile([P, NT], F16A, tag="x")
                    nc.sync.dma_start(
                        xt[:], xb[kc * P:(kc + 1) * P, t * NT:(t + 1) * NT])
                    xs.append(xt)
                for kd in range(KD):
                    wp = wps.tile([P, NT], F32, tag="wps")
                    for kc in range(KD):
                        nc.tensor.matmul(
                            wp[:],
                            lhsT=qkvwT_sb[kc][:, kd * P:(kd + 1) * P],
                            rhs=xs[kc][:],
                            start=(kc == 0), stop=(kc == KD - 1))
                    w16 = w_store[kd][:, t * NT:(t + 1) * NT]
                    nc.scalar.activation(w16, wp[:], AF.Copy)
                    acc = norm2_part[:, kd * NTILES + t:kd * NTILES + t + 1]
                    if kd == 0:
                        sq0 = sqscr.tile([P, NT], F16, tag="sqscr")
                        nc.scalar.activation(sq0[:], wp[:], AF.Square,
                                             accum_out=acc)
                    else:
                        nc.vector.scalar_tensor_tensor(
                            out=wp[:], in0=w16, scalar=1.0, in1=w16,
                            op0=ALU.mult, op1=ALU.mult, accum_out=acc)

        # --- finalize norm2 -> invnorm2*LM_SCALE -> logits lhsT ------------
        nc.vector.memset(lmat[:], 0.0)
        for kd in range(KD):
            nc.vector.tensor_reduce(
                inv2[:, kd:kd + 1],
                norm2_part[:, kd * NTILES:(kd + 1) * NTILES],
                axis=mybir.AxisListType.X, op=ALU.add)
        nc.vector.reciprocal(inv2[:], inv2[:])
        nc.vector.tensor_scalar_mul(inv2[:], inv2[:], LM_SCALE)
        for kd in range(KD):
            # head 2*kd lives on partitions 0..63, head 2*kd+1 on 64..127
            nc.vector.tensor_copy(
                lmat[0:HD, kd * HEADS + 2 * kd:kd * HEADS + 2 * kd + 1],
                inv2[0:HD, kd:kd + 1])
            nc.vector.tensor_copy(
                lmat[HD:P, kd * HEADS + 2 * kd + 1:kd * HEADS + 2 * kd + 2],
                inv2[HD:P, kd:kd + 1])

        # =================== Phase 2: softmax over heads + dots ============
        with ExitStack() as p2:
            sqpool = p2.enter_context(tc.tile_pool(name="sq", bufs=8))
            hpool = p2.enter_context(tc.tile_pool(name="hp", bufs=4))
            scr = p2.enter_context(tc.tile_pool(name="scr", bufs=2))
            lps = p2.enter_context(tc.tile_pool(name="lps", bufs=3, space="PSUM"))
            sps = p2.enter_context(tc.tile_pool(name="sps", bufs=2, space="PSUM"))
            bps = p2.enter_context(tc.tile_pool(name="bps", bufs=3, space="PSUM"))
            for t in range(NTILES):
                sqs = []
                for kd in range(KD):
                    sq = sqpool.tile([P, NT], F16, tag="sq")
                    w16 = w_store[kd][:, t * NT:(t + 1) * NT]
                    if kd < 3:
                        nc.gpsimd.tensor_tensor(sq[:], w16, w16, op=ALU.mult)
                    else:
                        nc.scalar.activation(sq[:], w16, AF.Square)
                    sqs.append(sq)
                lg = lps.tile([HEADS, NT], F32, tag="lps")
                for i, kd in enumerate(range(KD)):
                    nc.tensor.matmul(
                        lg[:],
                        lhsT=lmat[:, kd * HEADS:(kd + 1) * HEADS],
                        rhs=sqs[kd][:],
                        start=(i == 0), stop=(i == KD - 1))
                p16 = hpool.tile([HEADS, NT], F16A, tag="p16")
                nc.scalar.activation(p16[:], lg[:], AF.Exp,
                                     scale=temp_sb[:, 0:1])
                smps = sps.tile([HEADS, NT], F32, tag="sps")
                nc.tensor.matmul(smps[:], lhsT=ones8_sb[:], rhs=p16[:])
                lns = hpool.tile([HEADS, NT], F32, tag="lns")
                nc.scalar.activation(lns[:], smps[:], AF.Ln)
                pre = hpool.tile([HEADS, NT], F32, tag="pre")
                nc.vector.scalar_tensor_tensor(
                    out=pre[:], in0=lg[:], scalar=temp_sb[:, 0:1],
                    in1=lns[:], op0=ALU.mult, op1=ALU.subtract)
                pi_t = pi_store[:, t * NT:(t + 1) * NT]
                nc.scalar.activation(pi_t, pre[:], AF.Exp,
                                     accum_out=s_part[:, t:t + 1])
                for kd in range(KD):
                    pib = bps.tile([P, NT], F32, tag="bps")
                    nc.tensor.matmul(
                        pib[:], lhsT=ind_sb[:, kd * P:(kd + 1) * P], rhs=pi_t)
                    tscr = scr.tile([P, NT], F16, tag="tscr")
                    nc.vector.scalar_tensor_tensor(
                        out=tscr[:], in0=sqs[kd][:], scalar=1.0,
                        in1=pib[:], op0=ALU.mult, op1=ALU.mult,
                        accum_out=dots_part[:, kd * NTILES + t:
                                            kd * NTILES + t + 1])

            # --- finalize: S, dots, attn, fold -attn into outwT ------------
            svec = hpool.tile([HEADS, 1], F32, tag="svec")
            nc.vector.tensor_reduce(svec[:], s_part[:],
                                    axis=mybir.AxisListType.X, op=ALU.add)
            nc.vector.tensor_scalar_add(svec[:], svec[:], 1e-8)
            nc.vector.reciprocal(svec[:], svec[:])
            # bounce 1/(S+eps) through DRAM to broadcast head values to the
            # per-d partition layout (engines can't shift partitions).
            nc.sync.dma_start(svec_dram, svec[:])
            srb = hpool.tile([P, KD], F32, tag="srb")
            for kd in range(KD):
                src = (svec_dram[2 * kd:2 * kd + 2, :]
                       .rearrange("h (r one) -> h r one", r=1)
                       .broadcast_to([2, HD, 1]))
                nc.sync.dma_start(srb[:, kd:kd + 1], src)
                dk = nattn[:, kd:kd + 1]
                nc.vector.tensor_reduce(
                    dk, dots_part[:, kd * NTILES:(kd + 1) * NTILES],
                    axis=mybir.AxisListType.X, op=ALU.add)
                # dots_n = dots * (1/(S+eps)); attn = 1/(1+dots_n)
                nc.vector.tensor_scalar(
                    dk, dk, scalar1=srb[:, kd:kd + 1], scalar2=1.0,
                    op0=ALU.mult, op1=ALU.add)
                nc.vector.reciprocal(dk, dk)
                nc.vector.tensor_scalar_mul(dk, dk, -1.0)
                nc.vector.tensor_scalar(
                    outwA[kd][:], outwT_sb[kd][:], scalar1=dk,
                    scalar2=None, op0=ALU.mult)

        # =================== Phase 3: output + projection ==================
        with ExitStack() as p3:
            opool = p3.enter_context(tc.tile_pool(name="o", bufs=8))
            ypool = p3.enter_context(tc.tile_pool(name="y", bufs=8))
            b2ps = p3.enter_context(tc.tile_pool(name="b2ps", bufs=4, space="PSUM"))
            ops = p3.enter_context(tc.tile_pool(name="ops", bufs=4, space="PSUM"))
            for t in range(NTILES):
                os_ = []
                for kd in range(KD):
                    pib = b2ps.tile([P, NT], F32, tag="b2ps")
                    nc.tensor.matmul(
                        pib[:], lhsT=ind_sb[:, kd * P:(kd + 1) * P],
                        rhs=pi_store[:, t * NT:(t + 1) * NT])
                    ot = opool.tile([P, NT], F16A, tag="o")
                    w16 = w_store[kd][:, t * NT:(t + 1) * NT]
                    nc.vector.tensor_mul(ot[:], w16, pib[:])
                    os_.append(ot)
                for kc in range(KD):
                    yp = ops.tile([P, NT], F32, tag="ops")
                    for kd in range(KD):
                        nc.tensor.matmul(
                            yp[:],
                            lhsT=outwA[kd][:, kc * P:(kc + 1) * P],
                            rhs=os_[kd][:],
                            start=(kd == 0), stop=(kd == KD - 1))
                    yt = ypool.tile([P, NT], F32, tag="y")
                    nc.scalar.activation(yt[:], yp[:], AF.Identity,
                                         bias=outb_sb[:, kc:kc + 1],
                                         scale=1.0)
                    nc.sync.dma_start(
                        y[kc * P:(kc + 1) * P, t * NT:(t + 1) * NT], yt[:])

    nc.compile()
    _dedupe_act_table_loads(nc)
    return nc


def _host_inputs(x, qkv_w, temp, out_w, out_b):
    n_tokens = x.shape[2] * x.shape[3]
    qkvwT = np.ascontiguousarray(np.asarray(qkv_w).T).astype(np.float16)
    outwT = np.ascontiguousarray(np.asarray(out_w).T).astype(np.float16)
    ind = np.zeros((HEADS, C), np.float16)
    for d in range(C):
        ind[d // HD, d] = 1.0
    ones8 = np.ones((HEADS, HEADS), np.float16)
    temp_sc = (np.asarray(temp, np.float32) / LM_SCALE).reshape(HEADS, 1)
    outb_a = np.asarray(out_b, np.float32).reshape(C, 1)
    maps = []
    for i in range(x.shape[0]):
        maps.append({
            "xb": np.asarray(x[i], np.float32).reshape(C, n_tokens)
            .astype(np.float16),
            "qkvwT": qkvwT, "outwT": outwT, "ind": ind, "ones8": ones8,
            "temp_s": temp_sc, "outb": outb_a,
        })
    return maps


def kernel(x, qkv_w, temp, out_w, out_b):
    x = np.asarray(x)
    b, c, h, w = x.shape
    n_tokens = h * w
    key = (n_tokens, b)
    if key not in _NC_CACHE:
        _NC_CACHE[key] = _build_nc(n_tokens=n_tokens, n_cores=b)
    nc = _NC_CACHE[key]
    in_maps = _host_inputs(x, qkv_w, temp, out_w, out_b)
    res = run_bass_kernel_spmd(nc, in_maps, list(range(b)))
    out = np.stack([res.results[i]["y"].reshape(c, h, w) for i in range(b)])
    return out.astype(np.float32)

